# revision 8
# baseline (speedup 1.0000x reference)
"""BNAF forward (B=2048, D=8, H=512, 4 masked layers) on 8 TRN2 NeuronCores.

Strategy
--------
Pure data parallel: batch is split 256/core; the small weights are replicated.

Math: the BNAF log-det recursion collapses in exp space.  For each masked
linear layer, exp(logdet diag blocks) == the diag blocks of the normalized
weight w itself, and for tanh, exp(logdet) == 1 - h^2.  So the whole
log-sum-exp flow is a chain of *positive* block-diagonal matmuls with one
log() at the very end.  The per-output norm scale s = exp(logg)/||v|| is
folded into the G-flow moving operand (G_in = s * G), so the G-flow
stationary is just exp(W) block-diag.

v2 layout/schedule rewrite vs the original baseline:
- weights land via TWO packed SWDGE cast-DMAs (one [128,1280] f32->fp16 per
  big layer, diag tiles first then the strictly-lower windows), issued on
  the Q7 before anything else touches it.  No more 8 serialized window DMAs.
- no dummy-PE-warmup spam; a short burst only.
- diag exp is ONE ACT op per layer over the contiguous diag strip, into the
  G-flow stationary; quadrant fixes are gpsimd memsets/copies.
- per-out-feature norms: ones-stationary window matmuls -> [4,512] psum row,
  gpsimd per-quarter gather -> ONE PE transpose -> [128,4], DVE Newton rsqrt.
  (replaces 8 LDW+MM pairs of 1x128x1 transposes per layer)
- layer 4 runs fully transposed (stationary = h3/G3 batch-halves, moving =
  the tiny [128,8] layer-4 weights) producing [128,16] tiles: batch on
  partitions, so the tail elementwise chain + final log run on 128 lanes.
- final Ln is a 2-op DVE fast-log (bitcast + affine), no second ACT table.
- ACT only ever uses {Exp, Tanh}: single table load at kernel start.
"""

import numpy as np

TRACE = False          # set by test.py for profiling runs
LAST_RESULTS = None    # BassKernelResults stash for test.py

_CACHE = {}

P = 128
BC = 256          # batch per core
H = 512
NCORE = 8
MAGIC = 0x5f3759df
OFF2 = (512, 896, 1152)   # packed col offsets of the strictly-lower windows
LN2_A = 8.262958294867817e-08     # ln2 * 2^-23
LN2_B = -87.99988660234897        # -126.9570 * ln2

# smalls layout: first the exp block (exp'd in one ACT op), then the rest
_SM = {}
_off = 0
for _name, _w in [("w1dg", 4), ("lg1", 4), ("lg2", 4), ("lg3", 4),
                  ("lg4r", 8), ("w1n", 32), ("w4t", 32),          # exp block
                  ("b1", 4), ("b2", 4), ("b3", 4), ("b4rep", 16),
                  ("md1n", 32), ("mo1n", 32), ("md4t", 32), ("mo4t", 32),
                  ("ident", 128)]:
    _SM[_name] = (_off, _off + _w)
    _off += _w
SMALL_W = _off
EXPW = _SM["w4t"][1]          # width of the exp block (88)


def _vsl(vt, k, c):
    """Packed-layout slice of the (in-chunk k, out-chunk c) 128x128 block."""
    if k == c:
        return vt[:, 128 * k:128 * k + 128]
    o = OFF2[k] + 128 * (c - k - 1)
    return vt[:, o:o + 128]


def _build():
    import concourse.bacc as bacc
    import concourse.mybir as mybir
    import concourse.tile as tile
    from contextlib import ExitStack

    f32 = mybir.dt.float32
    u32 = mybir.dt.uint32
    bf16 = mybir.dt.bfloat16
    fp16 = mybir.dt.float16
    E = mybir.ActivationFunctionType
    ALU = mybir.AluOpType

    nc = bacc.Bacc("TRN2", target_bir_lowering=False, debug=False,
                   enable_asserts=False, num_devices=NCORE)

    t = {}
    t["xT"] = nc.dram_tensor("xT", (8, BC), f32, kind="ExternalInput").ap()
    t["wp2"] = nc.dram_tensor("wp2", (P, 1280), f32, kind="ExternalInput").ap()
    t["wp3"] = nc.dram_tensor("wp3", (P, 1280), f32, kind="ExternalInput").ap()
    t["smalls"] = nc.dram_tensor("smalls", (P, SMALL_W), f32, kind="ExternalInput").ap()
    t["h4T_out"] = nc.dram_tensor("h4T_out", (P, 16), f32, kind="ExternalOutput").ap()
    t["sldT_out"] = nc.dram_tensor("sldT_out", (P, 16), f32, kind="ExternalOutput").ap()

    def mm(out, lhsT, rhs, **kw):
        nc.tensor.matmul(out, lhsT, rhs, **kw)

    with tile.TileContext(nc) as tc, ExitStack() as ctx:
        wgt = ctx.enter_context(tc.tile_pool(name="wgt", bufs=1))
        scr = ctx.enter_context(tc.tile_pool(name="scr", bufs=3))
        pz = ctx.enter_context(tc.tile_pool(name="pz", bufs=2, space="PSUM"))
        pf = ctx.enter_context(tc.tile_pool(name="pf", bufs=1, space="PSUM"))
        pn = ctx.enter_context(tc.tile_pool(name="pn", bufs=1, space="PSUM"))

        act = nc.scalar.activation
        cp = nc.vector.tensor_copy
        ts = nc.vector.tensor_scalar
        stt = nc.vector.scalar_tensor_tensor
        mul = nc.vector.tensor_mul
        tt = nc.vector.tensor_tensor
        gcp = nc.gpsimd.tensor_copy
        gms = nc.gpsimd.memset

        # ---- DMAs first: Q7 issues the big casts before anything else ----
        xTt = wgt.tile([8, BC], fp16, name="xTt", tag="xTt")
        nc.gpsimd.dma_start(xTt, t["xT"])
        vt2 = wgt.tile([P, 1280], fp16, name="vt2", tag="vt2")
        nc.gpsimd.dma_start(vt2, t["wp2"])
        vt3 = wgt.tile([P, 1280], fp16, name="vt3", tag="vt3")
        nc.gpsimd.dma_start(vt3, t["wp3"])
        smalls = wgt.tile([P, SMALL_W], f32, name="smalls_t", tag="smalls_t")
        nc.sync.dma_start(smalls, t["smalls"])

        def sm(name):
            a, b = _SM[name]
            return smalls[:, a:b]

        ident = sm("ident")

        # ---- tiny constants on DVE (keep Q7 free) ----
        magict = wgt.tile([P, 8], u32, name="magict", tag="magict")
        nc.vector.memset(magict, MAGIC)
        ones4f = wgt.tile([P, 4], f32, name="ones4f", tag="ones4f")
        nc.vector.memset(ones4f, 1.0)
        ones4 = wgt.tile([P, 4], fp16, name="ones4", tag="ones4")
        cp(ones4, ones4f)
        wz = wgt.tile([P, BC], fp16, name="wz", tag="wz")
        nc.vector.memset(wz, 0.0)

        # short PE warm-up burst (HAM un-throttle) while DMAs drain
        pw = pn.tile([2, BC - 2], f32, name="pw", tag="pn")
        for _ in range(8):
            mm(pw, wz[:, 0:2], wz[:, 2:BC], skip_group_check=True)

        # one batched exp over the whole exp block
        esm = wgt.tile([P, EXPW], f32, name="esm", tag="esm")
        act(esm, smalls[:, 0:EXPW], E.Exp)

        def esl(name):
            a, b = _SM[name]
            return esm[:, a:b]

        e1d = esl("w1dg")
        eg = {1: esl("lg1"), 2: esl("lg2"), 3: esl("lg3")}
        eg4row = esm[0:1, _SM["lg4r"][0]:_SM["lg4r"][0] + 8]
        e1n = esl("w1n")
        e4t = esl("w4t")

        # s = eg * rsqrt(norm2): DVE-only Newton rsqrt
        def make_scale(n2_ap, eg_ap, shape, nm):
            pr = shape[0]
            n2s = scr.tile(list(shape), f32, name=f"n2s_{nm}", tag="sc_n2s")
            cp(n2s, n2_ap)
            shf = scr.tile(list(shape), u32, name=f"shf_{nm}", tag="sc_shf")
            ts(shf, n2s.bitcast(u32), 1, None, op0=ALU.arith_shift_right)
            y0 = scr.tile(list(shape), u32, name=f"y0_{nm}", tag="sc_y0")
            stt(y0, magict[:pr, :shape[1]], 0, shf, op0=ALU.bypass, op1=ALU.subtract)
            y = y0.bitcast(f32)
            t1 = scr.tile(list(shape), f32, name=f"t1_{nm}", tag="sc_t1")
            t2 = scr.tile(list(shape), f32, name=f"t2_{nm}", tag="sc_t2")
            for it in range(2):         # two Newton steps: y *= 1.5 - 0.5*n2*y*y
                mul(t1, y, y)
                mul(t2, t1, n2s)
                ts(t1, t2, -0.5, 1.5, op0=ALU.mult, op1=ALU.add)
                yn = scr.tile(list(shape), f32, name=f"yn{it}_{nm}", tag=f"sc_yn{it}")
                mul(yn, y, t1)
                y = yn
            s = wgt.tile(list(shape), f32, name=f"s_{nm}", tag=f"s_{nm}")
            mul(s, eg_ap, y)
            return s

        # ================= layer 1 prep (natural layout [512,8]) =========
        v1n = wgt.tile([P, 32], f32, name="v1n", tag="v1n")
        n1 = wgt.tile([P, 4], f32, name="n1", tag="n1")
        vT1 = wgt.tile([8, H], fp16, name="vT1", tag="vT1")
        v1a = scr.tile([P, 32], f32, name="v1a", tag="v1a")
        mul(v1a, e1n, sm("md1n"))
        v1b = scr.tile([P, 32], f32, name="v1b", tag="v1b")
        mul(v1b, sm("w1n"), sm("mo1n"))
        tt(v1n, v1a, v1b, op=ALU.add)
        for c in range(4):
            sq1 = scr.tile([P, 8], f32, name=f"sq1_{c}", tag="sq1")
            stt(sq1, v1n[:, 8 * c:8 * c + 8], 0, v1n[:, 8 * c:8 * c + 8],
                op0=ALU.bypass, op1=ALU.mult, accum_out=n1[:, c:c + 1])
        s1 = make_scale(n1, eg[1], (P, 4), "l1")
        e1s = wgt.tile([P, 4], f32, name="e1s", tag="e1s")
        mul(e1s, e1d, s1)
        for c in range(4):
            pt = pf.tile([8, P], f32, name=f"pt1_{c}", tag="pt1")
            nc.tensor.transpose(pt, v1n[:, 8 * c:8 * c + 8], ident)
            cp(vT1[:, P * c:P * c + P], pt)

        # ======= layer 4 prep (early: only needs smalls/esm) ==============
        v4a = scr.tile([P, 32], f32, name="v4a", tag="v4a")
        mul(v4a, e4t, sm("md4t"))
        v4b = scr.tile([P, 32], f32, name="v4b", tag="v4b")
        mul(v4b, sm("w4t"), sm("mo4t"))
        vt4 = wgt.tile([P, 32], fp16, name="vt4", tag="vt4")
        tt(vt4, v4a, v4b, op=ALU.add)
        vsq4 = scr.tile([P, 32], fp16, name="vsq4", tag="vsq4")
        mul(vsq4, vt4, vt4)
        n4 = pn.tile([1, 8], f32, name="n4", tag="pn")
        for k in range(4):
            mm(n4, ones4[:, 0:1], vsq4[:, 8 * k:8 * k + 8],
               start=(k == 0), stop=(k == 3))
        s4r = make_scale(n4, eg4row, (1, 8), "l4")
        s4b = wgt.tile([P, 16], f32, name="s4b", tag="s4b")
        nc.gpsimd.partition_broadcast(s4b[:, 0:8], s4r)
        gcp(s4b[:, 8:16], s4b[:, 0:8])
        # G-flow layer-4 stationary: exp(W4 diag blocks); s3 already in G3
        vd4 = wgt.tile([P, 32], bf16, name="vd4", tag="vd4")
        cp(vd4, v4a)

        # ================= layer 1 batch ==================================
        h1 = wgt.tile([P, 4 * BC], fp16, name="h1", tag="h1")
        pz1 = pz.tile([P, 4 * BC], f32, name="pz1", tag="pz")
        for c in range(4):
            mm(pz1[:, BC * c:BC * c + BC], vT1[:, P * c:P * c + P], xTt)
            act(h1[:, BC * c:BC * c + BC], pz1[:, BC * c:BC * c + BC], E.Tanh,
                bias=sm("b1")[:, c:c + 1], scale=s1[:, c:c + 1])
        hq1 = scr.tile([P, 4 * BC], fp16, name="hq1", tag="hq")
        mul(hq1, h1, h1)
        sc1 = scr.tile([P, 4 * BC], fp16, name="sc1", tag="sech2")
        ts(sc1, hq1, -1.0, 1.0, op0=ALU.mult, op1=ALU.add)
        G1 = wgt.tile([P, 4 * BC], bf16, name="G1", tag="G1")
        for c in range(4):
            ts(G1[:, BC * c:BC * c + BC], sc1[:, BC * c:BC * c + BC],
               e1s[:, c:c + 1], None, op0=ALU.mult)

        # ================= layer 2/3 prep ================================
        def prep_stat(l, vt):
            # G-flow stationary: exp of the whole diag strip in ONE ACT op
            Gd = wgt.tile([P, H], bf16, name=f"Gd{l}", tag=f"Gd{l}")
            act(Gd, vt[:, 0:H], E.Exp)
            for c in range(4):
                F = P * c
                # zero Gd's off-diagonal 64x64 quadrants (UR junk / LL junk)
                gms(Gd[0:64, F + 64:F + 128], 0.0)
                gms(Gd[64:128, F:F + 64], 0.0)
                # h-flow diag fix: copy exp'd 64x64 blocks back, zero LL
                gcp(vt[0:64, F:F + 64], Gd[0:64, F:F + 64])
                gcp(vt[64:128, F + 64:F + 128], Gd[64:128, F + 64:F + 128])
                gms(vt[64:128, F:F + 64], 0.0)
            # vsq over the whole packed tile (after diag fix)
            vsq = scr.tile([P, 1280], fp16, name=f"vsq{l}", tag="vsq")
            mul(vsq, vt, vt)
            return Gd, vsq

        def prep_norms(l, vsq):
            # norm2 row: ones-stationary matmuls -> [1, 512] psum
            nrow = pn.tile([1, H], f32, name=f"nrow{l}", tag="pn")
            mm(nrow, ones4[:, 0:1], vsq[:, 0:H], start=True, stop=False,
               skip_group_check=True)
            for k in range(3):
                w = 384 - 128 * k
                mm(nrow[:, P * (k + 1):H], ones4[:, 0:1],
                   vsq[:, OFF2[k]:OFF2[k] + w],
                   start=False, stop=(k == 2), skip_group_check=True)
            nrS = scr.tile([1, H], f32, name=f"nrS{l}", tag="nrS")
            cp(nrS, nrow)
            # columnize: four tiny PE transposes [1,128] -> [128,1]
            tp = pn.tile([P, 4], f32, name=f"tp{l}", tag="pn")
            for q in range(4):
                nc.tensor.transpose(tp[:, q:q + 1], nrS[0:1, P * q:P * q + P],
                                    ident[0:1, 0:1])
            return make_scale(tp, eg[l], (P, 4), f"l{l}")

        Gd2, vsq2 = prep_stat(2, vt2)
        s2 = prep_norms(2, vsq2)

        # ================= layer 2/3 batch ================================
        def big_batch(l, vt, Gd, s, h_prev, G_prev):
            hl = wgt.tile([P, 4 * BC], fp16, name=f"h{l}", tag=f"h{l}")
            pzl = pz.tile([P, 4 * BC], f32, name=f"pz{l}", tag="pz")
            pfl = pf.tile([P, 4 * BC], f32, name=f"pf{l}", tag="pf")
            for c in range(4):
                zc = pzl[:, BC * c:BC * c + BC]
                for k in range(c + 1):
                    mm(zc, _vsl(vt, k, c), h_prev[:, BC * k:BC * k + BC],
                       start=(k == 0), stop=(k == c))
                act(hl[:, BC * c:BC * c + BC], zc, E.Tanh,
                    bias=sm(f"b{l}")[:, c:c + 1], scale=s[:, c:c + 1])
                mm(pfl[:, BC * c:BC * c + BC], Gd[:, P * c:P * c + P],
                   G_prev[:, BC * c:BC * c + BC])
            hql = scr.tile([P, 4 * BC], fp16, name=f"hq{l}", tag="hq")
            nc.gpsimd.tensor_mul(hql, hl, hl)
            scl = scr.tile([P, 4 * BC], fp16, name=f"sc{l}", tag="sech2")
            nc.gpsimd.tensor_scalar(scl, hql, -1.0, 1.0, op0=ALU.mult, op1=ALU.add)
            Gl = wgt.tile([P, 4 * BC], bf16, name=f"G{l}", tag=f"G{l}")
            for c in range(4):
                stt(Gl[:, BC * c:BC * c + BC], pfl[:, BC * c:BC * c + BC],
                    s[:, c:c + 1], scl[:, BC * c:BC * c + BC],
                    op0=ALU.mult, op1=ALU.mult)
            return hl, Gl

        Gd3, vsq3 = prep_stat(3, vt3)

        h2, G2 = big_batch(2, vt2, Gd2, s2, h1, G1)

        s3 = prep_norms(3, vsq3)

        h3, G3 = big_batch(3, vt3, Gd3, s3, h2, G2)

        # ================= layer 4 batch (fully transposed) ===============
        z4 = pn.tile([P, 16], f32, name="z4", tag="pn")
        for b in range(2):
            for k in range(4):
                mm(z4[:, 8 * b:8 * b + 8],
                   h3[:, BC * k + P * b:BC * k + P * b + P],
                   vt4[:, 8 * k:8 * k + 8], start=(k == 0), stop=(k == 3))
        # scale/bias in the transposed layout (per free-col): z*s4 + b4
        z4s = scr.tile([P, 16], f32, name="z4s", tag="z4s")
        mul(z4s, z4, s4b)
        tt(z4s, z4s, sm("b4rep"), op=ALU.add)
        h4 = wgt.tile([P, 16], f32, name="h4", tag="h4")
        act(h4, z4s, E.Tanh)
        nc.sync.dma_start(t["h4T_out"], h4)
        p4 = pf.tile([P, 16], f32, name="p4", tag="pf")
        for b in range(2):
            for k in range(4):
                mm(p4[:, 8 * b:8 * b + 8],
                   G3[:, BC * k + P * b:BC * k + P * b + P],
                   vd4[:, 8 * k:8 * k + 8], start=(k == 0), stop=(k == 3))
        hq4 = scr.tile([P, 16], f32, name="hq4", tag="hq4")
        mul(hq4, h4, h4)
        s24 = scr.tile([P, 16], f32, name="s24", tag="s24")
        ts(s24, hq4, -1.0, 1.0, op0=ALU.mult, op1=ALU.add)
        # gt = s4 * p4 * (1 - h4^2); all positive
        gp = scr.tile([P, 16], f32, name="gp", tag="gp")
        mul(gp, p4, s4b)
        gt = wgt.tile([P, 16], f32, name="gt", tag="gt")
        mul(gt, gp, s24)
        # fast log: ln(x) ~= LN2_A * float(bits(x)) + LN2_B
        gf = scr.tile([P, 16], f32, name="gf", tag="gf")
        cp(gf, gt.bitcast(u32))
        sld = wgt.tile([P, 16], f32, name="sld", tag="sld")
        ts(sld, gf, LN2_A, LN2_B, op0=ALU.mult, op1=ALU.add)
        nc.sync.dma_start(t["sldT_out"], sld)

    nc.compile()
    return nc


def _host_prep(x, W1, logg1, bias1, W2, logg2, bias2, W3, logg3, bias3,
               W4, logg4, bias4):
    """Pure layout prep (transpose / reshape / gather / masks), no arithmetic."""
    f = np.float32

    def cols(a):          # [512]-ish vector -> [128, 4] column-chunk layout
        return np.ascontiguousarray(np.reshape(a, (4, P)).T).astype(f)

    def fold(m):          # [512, 8] -> [128, (k x)] with k = row-chunk
        return m.reshape(4, P, 8).transpose(1, 0, 2).reshape(P, 32)

    def pack(WT):         # [512, 512] W.T -> [128, 1280] diag strip + windows
        wp = np.empty((P, 1280), f)
        for k in range(4):
            wp[:, P * k:P * k + P] = WT[P * k:P * k + P, P * k:P * k + P]
        for k in range(3):
            w = 384 - 128 * k
            wp[:, OFF2[k]:OFF2[k] + w] = WT[P * k:P * k + P, P * (k + 1):H]
        return wp

    smalls = np.zeros((P, SMALL_W), f)

    def put(name, arr):
        a, b = _SM[name]
        smalls[:arr.shape[0], a:b] = arr

    put("ident", np.eye(P, dtype=f))
    put("w1n", fold(np.asarray(W1)))                       # natural [512,8]
    put("w4t", fold(np.ascontiguousarray(np.asarray(W4).T)))  # [512,8]
    put("w1dg", cols(W1[np.arange(H), np.arange(H) // 64]))
    put("lg1", cols(logg1)); put("b1", cols(bias1))
    put("lg2", cols(logg2)); put("b2", cols(bias2))
    put("lg3", cols(logg3)); put("b3", cols(bias3))
    smalls[0, _SM["lg4r"][0]:_SM["lg4r"][0] + 8] = np.asarray(logg4).reshape(8)
    put("b4rep", np.broadcast_to(
        np.concatenate([np.asarray(bias4).reshape(8)] * 2).reshape(1, 16),
        (P, 16)))
    # structural masks
    o = np.arange(H)[:, None] // 64
    i1 = np.arange(8)[None, :]
    md1 = (i1 == o).astype(f); mo1 = (i1 < o).astype(f)    # [512, 8] natural
    put("md1n", fold(md1)); put("mo1n", fold(mo1))
    ii = np.arange(H)[:, None] // 64
    o4 = np.arange(8)[None, :]
    md4 = (o4 == ii).astype(f); mo4 = (o4 > ii).astype(f)  # [512, 8] W4.T
    put("md4t", fold(md4)); put("mo4t", fold(mo4))

    wp2 = pack(np.ascontiguousarray(np.asarray(W2).T).astype(f))
    wp3 = pack(np.ascontiguousarray(np.asarray(W3).T).astype(f))
    xT = np.ascontiguousarray(np.asarray(x).T).astype(f)   # [8, 2048]
    return xT, wp2, wp3, smalls


def kernel(**inputs):
    global LAST_RESULTS
    from concourse.bass_utils import run_bass_kernel_spmd

    xT, wp2, wp3, smalls = _host_prep(**{k: np.asarray(v) for k, v in inputs.items()})

    if "nc" not in _CACHE:
        _CACHE["nc"] = _build()
    nc = _CACHE["nc"]

    in_maps = []
    for c in range(NCORE):
        in_maps.append({
            "xT": np.ascontiguousarray(xT[:, BC * c:BC * (c + 1)]),
            "wp2": wp2, "wp3": wp3, "smalls": smalls,
        })
    res = run_bass_kernel_spmd(nc, in_maps, core_ids=list(range(NCORE)),
                               trace=TRACE)
    LAST_RESULTS = res

    B = BC * NCORE
    h = np.empty((B, 8), np.float32)
    sld = np.empty((B, 8), np.float32)
    for c, r in enumerate(res.results):
        h4 = r["h4T_out"]          # [128, 16]: h[128b+p, o] = h4[p, 8b+o]
        sl = r["sldT_out"]
        for b in range(2):
            h[BC * c + P * b: BC * c + P * (b + 1)] = h4[:, 8 * b:8 * b + 8]
            sld[BC * c + P * b: BC * c + P * (b + 1)] = sl[:, 8 * b:8 * b + 8]
    return h, sld


# revision 9
# speedup vs baseline: 1.1291x; 1.1291x over previous
"""BNAF forward (B=2048, D=8, H=512, 4 masked layers) on 8 TRN2 NeuronCores.

Strategy
--------
Pure data parallel: batch is split 256/core; the small weights are replicated.

Math: the BNAF log-det recursion collapses in exp space.  For each masked
linear layer, exp(logdet diag blocks) == the diag blocks of the normalized
weight w itself, and for tanh, exp(logdet) == 1 - h^2.  So the whole
log-sum-exp flow is a chain of *positive* block-diagonal matmuls with one
log() at the very end.  The per-output norm scale s = exp(logg)/||v|| is
folded into the G-flow moving operand (G_in = s * G), so the G-flow
stationary is just exp(W) block-diag.

v3 layout/schedule:
- big-layer weights land as packed [128,1280] f32 HWDGE DMAs (diag strip
  first: [[Wd_A, UR],[0, Wd_B]] per chunk with host-zeroed LL, then the
  strictly-lower windows), one per layer, on the two HW DGE rings.
- gpsimd only does 2 big fp32->fp16 casts + the hq/sech2 elementwise pairs
  (every op on the Q7 costs ~0.4us fixed, so no small ops there).
- diag exp is TWO in-place strided ACT ops per layer (dA strip, dB strip);
  the G-flow stationary is a bf16 cast of the exp'd diag strip with UR
  quadrants zeroed (4 DVE memsets).
- per-out-feature norms: ones-stationary window matmuls -> [1,512] psum row
  -> DVE copy -> 4 tiny PE transposes -> [128,4] -> DVE Newton rsqrt.
- layer 4 runs fully transposed (stationary = h3/G3 batch-halves, moving =
  the tiny [128,8] layer-4 weights) producing [128,16] tiles: batch on
  partitions, so the tail elementwise chain + final log run on 128 lanes.
- final Ln is a 2-op DVE fast-log (bitcast + affine), no second ACT table.
- ACT only ever uses {Exp, Tanh}: single table load at kernel start.
"""

import numpy as np

TRACE = False          # set by test.py for profiling runs
LAST_RESULTS = None    # BassKernelResults stash for test.py

_CACHE = {}

P = 128
BC = 256          # batch per core
H = 512
NCORE = 8
MAGIC = 0x5f3759df
OFF2 = (512, 896, 1152)   # packed col offsets of the strictly-lower windows
LN2_A = 8.262958294867817e-08     # ln2 * 2^-23
LN2_B = -87.99988660234897        # -126.9570 * ln2

# smalls layout: first the exp block (exp'd in one ACT op), then the rest
_SM = {}
_off = 0
for _name, _w in [("w1dg", 4), ("lg1", 4), ("lg2", 4), ("lg3", 4),
                  ("lg4r", 8), ("w1n", 32), ("w4t", 32),          # exp block
                  ("b1", 4), ("b2", 4), ("b3", 4), ("b4rep", 16),
                  ("md1n", 32), ("mo1n", 32), ("md4t", 32), ("mo4t", 32),
                  ("ident", 128)]:
    _SM[_name] = (_off, _off + _w)
    _off += _w
SMALL_W = _off
EXPW = _SM["w4t"][1]          # width of the exp block (88)


def _vsl(vt, k, c):
    """Packed-layout slice of the (in-chunk k, out-chunk c) 128x128 block."""
    if k == c:
        return vt[:, 128 * k:128 * k + 128]
    o = OFF2[k] + 128 * (c - k - 1)
    return vt[:, o:o + 128]


def _build():
    import concourse.bacc as bacc
    import concourse.mybir as mybir
    import concourse.tile as tile
    from contextlib import ExitStack

    f32 = mybir.dt.float32
    u32 = mybir.dt.uint32
    bf16 = mybir.dt.bfloat16
    fp16 = mybir.dt.float16
    E = mybir.ActivationFunctionType
    ALU = mybir.AluOpType

    nc = bacc.Bacc("TRN2", target_bir_lowering=False, debug=False,
                   enable_asserts=False, num_devices=NCORE)

    t = {}
    t["xT"] = nc.dram_tensor("xT", (8, BC), f32, kind="ExternalInput").ap()
    t["wp2"] = nc.dram_tensor("wp2", (P, 1280), f32, kind="ExternalInput").ap()
    t["wp3"] = nc.dram_tensor("wp3", (P, 1280), f32, kind="ExternalInput").ap()
    t["smalls"] = nc.dram_tensor("smalls", (P, SMALL_W), f32, kind="ExternalInput").ap()
    t["h4T_out"] = nc.dram_tensor("h4T_out", (P, 16), f32, kind="ExternalOutput").ap()
    t["sldT_out"] = nc.dram_tensor("sldT_out", (P, 16), f32, kind="ExternalOutput").ap()

    def mm(out, lhsT, rhs, **kw):
        nc.tensor.matmul(out, lhsT, rhs, **kw)

    with tile.TileContext(nc) as tc, ExitStack() as ctx:
        wgt = ctx.enter_context(tc.tile_pool(name="wgt", bufs=1))
        scr = ctx.enter_context(tc.tile_pool(name="scr", bufs=3))
        pz = ctx.enter_context(tc.tile_pool(name="pz", bufs=2, space="PSUM"))
        pf = ctx.enter_context(tc.tile_pool(name="pf", bufs=1, space="PSUM"))
        pn = ctx.enter_context(tc.tile_pool(name="pn", bufs=1, space="PSUM"))

        act = nc.scalar.activation
        cp = nc.vector.tensor_copy
        ts = nc.vector.tensor_scalar
        stt = nc.vector.scalar_tensor_tensor
        mul = nc.vector.tensor_mul
        tt = nc.vector.tensor_tensor

        # ---- input DMAs: all HWDGE (sync + scalar rings), f32, no casts ----
        smalls = wgt.tile([P, SMALL_W], f32, name="smalls_t", tag="smalls_t")
        nc.sync.dma_start(smalls, t["smalls"])
        xTf = wgt.tile([8, BC], f32, name="xTf", tag="xTf")
        nc.sync.dma_start(xTf, t["xT"])
        wr2 = wgt.tile([P, 1280], f32, name="wr2", tag="wr2")
        nc.sync.dma_start(wr2, t["wp2"])
        wr3 = wgt.tile([P, 1280], f32, name="wr3", tag="wr3")
        nc.scalar.dma_start(wr3, t["wp3"])

        def sm(name):
            a, b = _SM[name]
            return smalls[:, a:b]

        ident = sm("ident")

        # ---- tiny constants on DVE (keep Q7 free) ----
        magict = wgt.tile([P, 8], u32, name="magict", tag="magict")
        nc.vector.memset(magict, MAGIC)
        ones4f = wgt.tile([P, 4], f32, name="ones4f", tag="ones4f")
        nc.vector.memset(ones4f, 1.0)
        ones4 = wgt.tile([P, 4], fp16, name="ones4", tag="ones4")
        cp(ones4, ones4f)
        wz = wgt.tile([P, BC], fp16, name="wz", tag="wz")
        nc.vector.memset(wz, 0.0)

        # short PE warm-up burst (HAM un-throttle) while DMAs drain
        pw = pn.tile([2, BC - 2], f32, name="pw", tag="pn")
        for _ in range(8):
            mm(pw, wz[:, 0:2], wz[:, 2:BC], skip_group_check=True)

        # one batched exp over the whole exp block
        esm = wgt.tile([P, EXPW], f32, name="esm", tag="esm")
        act(esm, smalls[:, 0:EXPW], E.Exp)

        def esl(name):
            a, b = _SM[name]
            return esm[:, a:b]

        e1d = esl("w1dg")
        eg = {1: esl("lg1"), 2: esl("lg2"), 3: esl("lg3")}
        eg4row = esm[0:1, _SM["lg4r"][0]:_SM["lg4r"][0] + 8]
        e1n = esl("w1n")
        e4t = esl("w4t")

        # fp16 cast of x (DVE, cheap)
        xTt = wgt.tile([8, BC], fp16, name="xTt", tag="xTt")
        cp(xTt, xTf)

        # s = eg * rsqrt(norm2): DVE-only Newton rsqrt
        def make_scale(n2_ap, eg_ap, shape, nm):
            pr = shape[0]
            n2s = scr.tile(list(shape), f32, name=f"n2s_{nm}", tag="sc_n2s")
            cp(n2s, n2_ap)
            shf = scr.tile(list(shape), u32, name=f"shf_{nm}", tag="sc_shf")
            ts(shf, n2s.bitcast(u32), 1, None, op0=ALU.arith_shift_right)
            y0 = scr.tile(list(shape), u32, name=f"y0_{nm}", tag="sc_y0")
            stt(y0, magict[:pr, :shape[1]], 0, shf, op0=ALU.bypass, op1=ALU.subtract)
            y = y0.bitcast(f32)
            t1 = scr.tile(list(shape), f32, name=f"t1_{nm}", tag="sc_t1")
            t2 = scr.tile(list(shape), f32, name=f"t2_{nm}", tag="sc_t2")
            for it in range(2):         # two Newton steps: y *= 1.5 - 0.5*n2*y*y
                mul(t1, y, y)
                mul(t2, t1, n2s)
                ts(t1, t2, -0.5, 1.5, op0=ALU.mult, op1=ALU.add)
                yn = scr.tile(list(shape), f32, name=f"yn{it}_{nm}", tag=f"sc_yn{it}")
                mul(yn, y, t1)
                y = yn
            s = wgt.tile(list(shape), f32, name=f"s_{nm}", tag=f"s_{nm}")
            mul(s, eg_ap, y)
            return s

        # ================= layer 1 prep (natural layout [512,8]) =========
        v1n = wgt.tile([P, 32], f32, name="v1n", tag="v1n")
        n1 = wgt.tile([P, 4], f32, name="n1", tag="n1")
        vT1 = wgt.tile([8, H], fp16, name="vT1", tag="vT1")
        v1a = scr.tile([P, 32], f32, name="v1a", tag="v1a")
        mul(v1a, e1n, sm("md1n"))
        v1b = scr.tile([P, 32], f32, name="v1b", tag="v1b")
        mul(v1b, sm("w1n"), sm("mo1n"))
        tt(v1n, v1a, v1b, op=ALU.add)
        for c in range(4):
            sq1 = scr.tile([P, 8], f32, name=f"sq1_{c}", tag="sq1")
            stt(sq1, v1n[:, 8 * c:8 * c + 8], 0, v1n[:, 8 * c:8 * c + 8],
                op0=ALU.bypass, op1=ALU.mult, accum_out=n1[:, c:c + 1])
        s1 = make_scale(n1, eg[1], (P, 4), "l1")
        e1s = wgt.tile([P, 4], f32, name="e1s", tag="e1s")
        mul(e1s, e1d, s1)
        for c in range(4):
            pt = pf.tile([8, P], f32, name=f"pt1_{c}", tag="pt1")
            nc.tensor.transpose(pt, v1n[:, 8 * c:8 * c + 8], ident)
            cp(vT1[:, P * c:P * c + P], pt)

        # ======= layer 4 prep (early: only needs smalls/esm) ==============
        v4a = scr.tile([P, 32], f32, name="v4a", tag="v4a")
        mul(v4a, e4t, sm("md4t"))
        v4b = scr.tile([P, 32], f32, name="v4b", tag="v4b")
        mul(v4b, sm("w4t"), sm("mo4t"))
        vt4 = wgt.tile([P, 32], fp16, name="vt4", tag="vt4")
        tt(vt4, v4a, v4b, op=ALU.add)
        vsq4 = scr.tile([P, 32], fp16, name="vsq4", tag="vsq4")
        mul(vsq4, vt4, vt4)
        n4 = pn.tile([1, 8], f32, name="n4", tag="pn")
        for k in range(4):
            mm(n4, ones4[:, 0:1], vsq4[:, 8 * k:8 * k + 8],
               start=(k == 0), stop=(k == 3))
        s4r = make_scale(n4, eg4row, (1, 8), "l4")
        s4b = wgt.tile([P, 16], f32, name="s4b", tag="s4b")
        nc.gpsimd.partition_broadcast(s4b[:, 0:8], s4r)
        nc.gpsimd.partition_broadcast(s4b[:, 8:16], s4r)
        # G-flow layer-4 stationary: exp(W4 diag blocks); s3 already in G3
        vd4 = wgt.tile([P, 32], bf16, name="vd4", tag="vd4")
        cp(vd4, v4a)

        # ================= layer 1 batch ==================================
        h1 = wgt.tile([P, 4 * BC], fp16, name="h1", tag="h1")
        pz1 = pz.tile([P, 4 * BC], f32, name="pz1", tag="pz")
        for c in range(4):
            mm(pz1[:, BC * c:BC * c + BC], vT1[:, P * c:P * c + P], xTt)
            act(h1[:, BC * c:BC * c + BC], pz1[:, BC * c:BC * c + BC], E.Tanh,
                bias=sm("b1")[:, c:c + 1], scale=s1[:, c:c + 1])
        hq1 = scr.tile([P, 4 * BC], fp16, name="hq1", tag="hq")
        mul(hq1, h1, h1)
        sc1 = scr.tile([P, 4 * BC], fp16, name="sc1", tag="sech2")
        ts(sc1, hq1, -1.0, 1.0, op0=ALU.mult, op1=ALU.add)
        G1 = wgt.tile([P, 4 * BC], bf16, name="G1", tag="G1")
        for c in range(4):
            ts(G1[:, BC * c:BC * c + BC], sc1[:, BC * c:BC * c + BC],
               e1s[:, c:c + 1], None, op0=ALU.mult)

        # ================= layer 2/3 prep ================================
        def prep_stat(l, wr):
            # fp16 cast of the whole packed tile (gpsimd: one big op)
            vt = wgt.tile([P, 1280], fp16, name=f"vt{l}", tag=f"vt{l}")
            nc.gpsimd.tensor_copy(vt, wr)
            # in-place exp of the 8 diag 64x64 blocks: two strided ACT ops
            dA = vt[0:64, 0:H].rearrange("p (b c) -> p b c", c=128)[:, :, 0:64]
            act(dA, dA, E.Exp)
            dB = vt[64:128, 0:H].rearrange("p (b c) -> p b c", c=128)[:, :, 64:128]
            act(dB, dB, E.Exp)
            # G-flow stationary: bf16 copy of diag strip, UR quadrants zeroed
            Gd = wgt.tile([P, H], bf16, name=f"Gd{l}", tag=f"Gd{l}")
            cp(Gd, vt[:, 0:H])
            GdUR = Gd[0:64, :].rearrange("p (b c) -> p b c", c=128)[:, :, 64:128]
            nc.vector.memset(GdUR, 0.0)
            # vsq over the whole packed tile (diag strip is post-exp)
            vsq = scr.tile([P, 1280], fp16, name=f"vsq{l}", tag="vsq")
            mul(vsq, vt, vt)
            return vt, Gd, vsq

        def prep_norms(l, vsq):
            # norm2 row: ones-stationary matmuls -> [1, 512] psum
            nrow = pn.tile([1, H], f32, name=f"nrow{l}", tag="pn")
            mm(nrow, ones4[:, 0:1], vsq[:, 0:H], start=True, stop=False,
               skip_group_check=True)
            for k in range(3):
                w = 384 - 128 * k
                mm(nrow[:, P * (k + 1):H], ones4[:, 0:1],
                   vsq[:, OFF2[k]:OFF2[k] + w],
                   start=False, stop=(k == 2), skip_group_check=True)
            nrS = scr.tile([1, H], f32, name=f"nrS{l}", tag="nrS")
            cp(nrS, nrow)
            # columnize: four tiny PE transposes [1,128] -> [128,1]
            tp = pn.tile([P, 4], f32, name=f"tp{l}", tag="pn")
            for q in range(4):
                nc.tensor.transpose(tp[:, q:q + 1], nrS[0:1, P * q:P * q + P],
                                    ident[0:1, 0:1])
            return make_scale(tp, eg[l], (P, 4), f"l{l}")

        vt2, Gd2, vsq2 = prep_stat(2, wr2)
        s2 = prep_norms(2, vsq2)
        vt3, Gd3, vsq3 = prep_stat(3, wr3)

        # ================= layer 2/3 batch ================================
        def big_batch(l, vt, Gd, s, h_prev, G_prev):
            hl = wgt.tile([P, 4 * BC], fp16, name=f"h{l}", tag=f"h{l}")
            pzl = pz.tile([P, 4 * BC], f32, name=f"pz{l}", tag="pz")
            pfl = pf.tile([P, 4 * BC], f32, name=f"pf{l}", tag="pf")
            for c in range(4):
                zc = pzl[:, BC * c:BC * c + BC]
                for k in range(c + 1):
                    mm(zc, _vsl(vt, k, c), h_prev[:, BC * k:BC * k + BC],
                       start=(k == 0), stop=(k == c))
                act(hl[:, BC * c:BC * c + BC], zc, E.Tanh,
                    bias=sm(f"b{l}")[:, c:c + 1], scale=s[:, c:c + 1])
                mm(pfl[:, BC * c:BC * c + BC], Gd[:, P * c:P * c + P],
                   G_prev[:, BC * c:BC * c + BC])
            hql = scr.tile([P, 4 * BC], fp16, name=f"hq{l}", tag="hq")
            nc.gpsimd.tensor_mul(hql, hl, hl)
            scl = scr.tile([P, 4 * BC], fp16, name=f"sc{l}", tag="sech2")
            nc.gpsimd.tensor_scalar(scl, hql, -1.0, 1.0, op0=ALU.mult, op1=ALU.add)
            Gl = wgt.tile([P, 4 * BC], bf16, name=f"G{l}", tag=f"G{l}")
            for c in range(4):
                stt(Gl[:, BC * c:BC * c + BC], pfl[:, BC * c:BC * c + BC],
                    s[:, c:c + 1], scl[:, BC * c:BC * c + BC],
                    op0=ALU.mult, op1=ALU.mult)
            return hl, Gl

        h2, G2 = big_batch(2, vt2, Gd2, s2, h1, G1)

        s3 = prep_norms(3, vsq3)

        h3, G3 = big_batch(3, vt3, Gd3, s3, h2, G2)

        # ================= layer 4 batch (fully transposed) ===============
        z4 = pn.tile([P, 16], f32, name="z4", tag="pn")
        for b in range(2):
            for k in range(4):
                mm(z4[:, 8 * b:8 * b + 8],
                   h3[:, BC * k + P * b:BC * k + P * b + P],
                   vt4[:, 8 * k:8 * k + 8], start=(k == 0), stop=(k == 3))
        # scale/bias in the transposed layout (per free-col): z*s4 + b4
        z4s = scr.tile([P, 16], f32, name="z4s", tag="z4s")
        mul(z4s, z4, s4b)
        z4t = scr.tile([P, 16], f32, name="z4t", tag="z4t")
        tt(z4t, z4s, sm("b4rep"), op=ALU.add)
        h4 = wgt.tile([P, 16], f32, name="h4", tag="h4")
        act(h4, z4t, E.Tanh)
        nc.sync.dma_start(t["h4T_out"], h4)
        p4 = pf.tile([P, 16], f32, name="p4", tag="pf")
        for b in range(2):
            for k in range(4):
                mm(p4[:, 8 * b:8 * b + 8],
                   G3[:, BC * k + P * b:BC * k + P * b + P],
                   vd4[:, 8 * k:8 * k + 8], start=(k == 0), stop=(k == 3))
        hq4 = scr.tile([P, 16], f32, name="hq4", tag="hq4")
        mul(hq4, h4, h4)
        s24 = scr.tile([P, 16], f32, name="s24", tag="s24")
        ts(s24, hq4, -1.0, 1.0, op0=ALU.mult, op1=ALU.add)
        # gt = s4 * p4 * (1 - h4^2); all positive
        gp = scr.tile([P, 16], f32, name="gp", tag="gp")
        mul(gp, p4, s4b)
        gt = wgt.tile([P, 16], f32, name="gt", tag="gt")
        mul(gt, gp, s24)
        # fast log: ln(x) ~= LN2_A * float(bits(x)) + LN2_B
        gf = scr.tile([P, 16], f32, name="gf", tag="gf")
        cp(gf, gt.bitcast(u32))
        sld = wgt.tile([P, 16], f32, name="sld", tag="sld")
        ts(sld, gf, LN2_A, LN2_B, op0=ALU.mult, op1=ALU.add)
        nc.sync.dma_start(t["sldT_out"], sld)

    nc.compile()
    return nc


def _host_prep(x, W1, logg1, bias1, W2, logg2, bias2, W3, logg3, bias3,
               W4, logg4, bias4):
    """Pure layout prep (transpose / reshape / gather / masks), no arithmetic."""
    f = np.float32

    def cols(a):          # [512]-ish vector -> [128, 4] column-chunk layout
        return np.ascontiguousarray(np.reshape(a, (4, P)).T).astype(f)

    def fold(m):          # [512, 8] -> [128, (k x)] with k = row-chunk
        return m.reshape(4, P, 8).transpose(1, 0, 2).reshape(P, 32)

    def pack(WT):         # [512, 512] W.T -> [128, 1280] diag strip + windows
        wp = np.empty((P, 1280), f)
        for k in range(4):
            d = np.array(WT[P * k:P * k + P, P * k:P * k + P])
            d[64:128, 0:64] = 0.0          # structural mask: LL quadrant
            wp[:, P * k:P * k + P] = d
        for k in range(3):
            w = 384 - 128 * k
            wp[:, OFF2[k]:OFF2[k] + w] = WT[P * k:P * k + P, P * (k + 1):H]
        return wp

    smalls = np.zeros((P, SMALL_W), f)

    def put(name, arr):
        a, b = _SM[name]
        smalls[:arr.shape[0], a:b] = arr

    put("ident", np.eye(P, dtype=f))
    put("w1n", fold(np.asarray(W1)))                       # natural [512,8]
    put("w4t", fold(np.ascontiguousarray(np.asarray(W4).T)))  # [512,8]
    put("w1dg", cols(W1[np.arange(H), np.arange(H) // 64]))
    put("lg1", cols(logg1)); put("b1", cols(bias1))
    put("lg2", cols(logg2)); put("b2", cols(bias2))
    put("lg3", cols(logg3)); put("b3", cols(bias3))
    smalls[0, _SM["lg4r"][0]:_SM["lg4r"][0] + 8] = np.asarray(logg4).reshape(8)
    put("b4rep", np.broadcast_to(
        np.concatenate([np.asarray(bias4).reshape(8)] * 2).reshape(1, 16),
        (P, 16)))
    # structural masks
    o = np.arange(H)[:, None] // 64
    i1 = np.arange(8)[None, :]
    md1 = (i1 == o).astype(f); mo1 = (i1 < o).astype(f)    # [512, 8] natural
    put("md1n", fold(md1)); put("mo1n", fold(mo1))
    ii = np.arange(H)[:, None] // 64
    o4 = np.arange(8)[None, :]
    md4 = (o4 == ii).astype(f); mo4 = (o4 > ii).astype(f)  # [512, 8] W4.T
    put("md4t", fold(md4)); put("mo4t", fold(mo4))

    wp2 = pack(np.ascontiguousarray(np.asarray(W2).T).astype(f))
    wp3 = pack(np.ascontiguousarray(np.asarray(W3).T).astype(f))
    xT = np.ascontiguousarray(np.asarray(x).T).astype(f)   # [8, 2048]
    return xT, wp2, wp3, smalls


def kernel(**inputs):
    global LAST_RESULTS
    from concourse.bass_utils import run_bass_kernel_spmd

    xT, wp2, wp3, smalls = _host_prep(**{k: np.asarray(v) for k, v in inputs.items()})

    if "nc" not in _CACHE:
        _CACHE["nc"] = _build()
    nc = _CACHE["nc"]

    in_maps = []
    for c in range(NCORE):
        in_maps.append({
            "xT": np.ascontiguousarray(xT[:, BC * c:BC * (c + 1)]),
            "wp2": wp2, "wp3": wp3, "smalls": smalls,
        })
    res = run_bass_kernel_spmd(nc, in_maps, core_ids=list(range(NCORE)),
                               trace=TRACE)
    LAST_RESULTS = res

    B = BC * NCORE
    h = np.empty((B, 8), np.float32)
    sld = np.empty((B, 8), np.float32)
    for c, r in enumerate(res.results):
        h4 = r["h4T_out"]          # [128, 16]: h[128b+p, o] = h4[p, 8b+o]
        sl = r["sldT_out"]
        for b in range(2):
            h[BC * c + P * b: BC * c + P * (b + 1)] = h4[:, 8 * b:8 * b + 8]
            sld[BC * c + P * b: BC * c + P * (b + 1)] = sl[:, 8 * b:8 * b + 8]
    return h, sld


# revision 14
# speedup vs baseline: 1.4576x; 1.2909x over previous
"""BNAF forward (B=2048, D=8, H=512, 4 masked layers) on 8 TRN2 NeuronCores.

Strategy
--------
Pure data parallel: batch is split 256/core; the small weights are replicated.

Math: the BNAF log-det recursion collapses in exp space.  For each masked
linear layer, exp(logdet diag blocks) == the diag blocks of the normalized
weight w itself, and for tanh, exp(logdet) == 1 - h^2.  So the whole
log-sum-exp flow is a chain of *positive* block-diagonal matmuls with one
log() at the very end.  The per-output norm scale s = exp(logg)/||v|| is
folded into the G-flow moving operand (G_in = s * G), so the G-flow
stationary is just exp(W) block-diag.

v3 layout/schedule:
- big-layer weights land as packed [128,1280] f32 HWDGE DMAs (diag strip
  first: [[Wd_A, UR],[0, Wd_B]] per chunk with host-zeroed LL, then the
  strictly-lower windows), one per layer, on the two HW DGE rings.
- gpsimd only does 2 big fp32->fp16 casts + the hq/sech2 elementwise pairs
  (every op on the Q7 costs ~0.4us fixed, so no small ops there).
- diag exp is TWO in-place strided ACT ops per layer (dA strip, dB strip);
  the G-flow stationary is a bf16 cast of the exp'd diag strip with UR
  quadrants zeroed (4 DVE memsets).
- per-out-feature norms: ones-stationary window matmuls -> [1,512] psum row
  -> DVE copy -> 4 tiny PE transposes -> [128,4] -> DVE Newton rsqrt.
- layer 4 runs fully transposed (stationary = h3/G3 batch-halves, moving =
  the tiny [128,8] layer-4 weights) producing [128,16] tiles: batch on
  partitions, so the tail elementwise chain + final log run on 128 lanes.
- final Ln is a 2-op DVE fast-log (bitcast + affine), no second ACT table.
- ACT only ever uses {Exp, Tanh}: single table load at kernel start.
"""

import numpy as np

TRACE = False          # set by test.py for profiling runs
LAST_RESULTS = None    # BassKernelResults stash for test.py

_CACHE = {}

P = 128
BC = 256          # batch per core
H = 512
NCORE = 8
MAGIC = 0x5f3759df
OFF2 = (512, 896, 1152)   # packed col offsets of the strictly-lower windows
LN2_A = 8.262958294867817e-08     # ln2 * 2^-23
LN2_B = -87.99988660234897        # -126.9570 * ln2

# smalls layout: first the exp block (exp'd in one ACT op), then the rest
_SM = {}
_off = 0
for _name, _w in [("w1dg", 4), ("lg1", 4), ("lg2", 4), ("lg3", 4),
                  ("lg4r", 8), ("w1n", 32), ("w4t", 32),          # exp block
                  ("b1", 4), ("b2", 4), ("b3", 4), ("b4rep", 16),
                  ("md1n", 32), ("mo1n", 32), ("md4t", 32), ("mo4t", 32),
                  ("ident", 128)]:
    _SM[_name] = (_off, _off + _w)
    _off += _w
SMALL_W = _off
EXPW = _SM["w4t"][1]          # width of the exp block (88)


def _vsl(vt, k, c):
    """Packed-layout slice of the (in-chunk k, out-chunk c) 128x128 block."""
    if k == c:
        return vt[:, 128 * k:128 * k + 128]
    o = OFF2[k] + 128 * (c - k - 1)
    return vt[:, o:o + 128]


def _build():
    import concourse.bacc as bacc
    import concourse.mybir as mybir
    import concourse.tile as tile
    from contextlib import ExitStack

    f32 = mybir.dt.float32
    u32 = mybir.dt.uint32
    bf16 = mybir.dt.bfloat16
    fp16 = mybir.dt.float16
    E = mybir.ActivationFunctionType
    ALU = mybir.AluOpType

    nc = bacc.Bacc("TRN2", target_bir_lowering=False, debug=False,
                   enable_asserts=False, num_devices=NCORE)

    t = {}
    t["xT"] = nc.dram_tensor("xT", (8, BC), f32, kind="ExternalInput").ap()
    t["wp2"] = nc.dram_tensor("wp2", (P, 1280), f32, kind="ExternalInput").ap()
    t["wp3"] = nc.dram_tensor("wp3", (P, 1280), f32, kind="ExternalInput").ap()
    t["smalls"] = nc.dram_tensor("smalls", (P, SMALL_W), f32, kind="ExternalInput").ap()
    t["h4T_out"] = nc.dram_tensor("h4T_out", (P, 16), f32, kind="ExternalOutput").ap()
    t["sldT_out"] = nc.dram_tensor("sldT_out", (P, 16), f32, kind="ExternalOutput").ap()

    def mm(out, lhsT, rhs, **kw):
        nc.tensor.matmul(out, lhsT, rhs, **kw)

    with tile.TileContext(nc) as tc, ExitStack() as ctx:
        wgt = ctx.enter_context(tc.tile_pool(name="wgt", bufs=1))
        scr = ctx.enter_context(tc.tile_pool(name="scr", bufs=3))
        pz = ctx.enter_context(tc.tile_pool(name="pz", bufs=2, space="PSUM"))
        pf = ctx.enter_context(tc.tile_pool(name="pf", bufs=1, space="PSUM"))
        pn = ctx.enter_context(tc.tile_pool(name="pn", bufs=1, space="PSUM"))

        act = nc.scalar.activation
        cp = nc.vector.tensor_copy
        ts = nc.vector.tensor_scalar
        stt = nc.vector.scalar_tensor_tensor
        mul = nc.vector.tensor_mul
        tt = nc.vector.tensor_tensor

        # ---- input DMAs: smalls on HWDGE; x + packed weights as SWDGE
        # cast-DMAs (fp32 DRAM -> fp16 SBUF, cast inline in the SDMA) ----
        smalls = wgt.tile([P, SMALL_W], f32, name="smalls_t", tag="smalls_t")
        nc.sync.dma_start(smalls, t["smalls"])
        xTt = wgt.tile([8, BC], fp16, name="xTt", tag="xTt")
        nc.gpsimd.dma_start(xTt, t["xT"])
        vt2 = wgt.tile([P, 1280], fp16, name="vt2", tag="vt2")
        nc.gpsimd.dma_start(vt2, t["wp2"])
        vt3 = wgt.tile([P, 1280], fp16, name="vt3", tag="vt3")
        nc.gpsimd.dma_start(vt3, t["wp3"])

        def sm(name):
            a, b = _SM[name]
            return smalls[:, a:b]

        ident = sm("ident")

        # ---- tiny constants on DVE (keep Q7 free) ----
        magict = wgt.tile([P, 8], u32, name="magict", tag="magict")
        nc.vector.memset(magict, MAGIC)
        ones4f = wgt.tile([P, 4], f32, name="ones4f", tag="ones4f")
        nc.vector.memset(ones4f, 1.0)
        ones4 = wgt.tile([P, 4], fp16, name="ones4", tag="ones4")
        cp(ones4, ones4f)
        wz = wgt.tile([P, BC], fp16, name="wz", tag="wz")
        nc.vector.memset(wz, 0.0)

        # short PE warm-up burst (HAM un-throttle) while DMAs drain
        pw = pn.tile([2, BC - 2], f32, name="pw", tag="pn")
        for _ in range(8):
            mm(pw, wz[:, 0:2], wz[:, 2:BC], skip_group_check=True)

        # one batched exp over the whole exp block
        esm = wgt.tile([P, EXPW], f32, name="esm", tag="esm")
        act(esm, smalls[:, 0:EXPW], E.Exp)

        def esl(name):
            a, b = _SM[name]
            return esm[:, a:b]

        e1d = esl("w1dg")
        eg = {1: esl("lg1"), 2: esl("lg2"), 3: esl("lg3")}
        eg4row = esm[0:1, _SM["lg4r"][0]:_SM["lg4r"][0] + 8]
        e1n = esl("w1n")
        e4t = esl("w4t")

        # s = eg * rsqrt(norm2): DVE-only Newton rsqrt
        def make_scale(n2_ap, eg_ap, shape, nm):
            pr = shape[0]
            n2s = scr.tile(list(shape), f32, name=f"n2s_{nm}", tag="sc_n2s")
            cp(n2s, n2_ap)
            shf = scr.tile(list(shape), u32, name=f"shf_{nm}", tag="sc_shf")
            ts(shf, n2s.bitcast(u32), 1, None, op0=ALU.arith_shift_right)
            y0 = scr.tile(list(shape), u32, name=f"y0_{nm}", tag="sc_y0")
            stt(y0, magict[:pr, :shape[1]], 0, shf, op0=ALU.bypass, op1=ALU.subtract)
            y = y0.bitcast(f32)
            t1 = scr.tile(list(shape), f32, name=f"t1_{nm}", tag="sc_t1")
            t2 = scr.tile(list(shape), f32, name=f"t2_{nm}", tag="sc_t2")
            for it in range(2):         # two Newton steps: y *= 1.5 - 0.5*n2*y*y
                mul(t1, y, y)
                mul(t2, t1, n2s)
                ts(t1, t2, -0.5, 1.5, op0=ALU.mult, op1=ALU.add)
                yn = scr.tile(list(shape), f32, name=f"yn{it}_{nm}", tag=f"sc_yn{it}")
                mul(yn, y, t1)
                y = yn
            s = wgt.tile(list(shape), f32, name=f"s_{nm}", tag=f"s_{nm}")
            mul(s, eg_ap, y)
            return s

        # ================= layer 1 prep (natural layout [512,8]) =========
        v1n = wgt.tile([P, 32], f32, name="v1n", tag="v1n")
        n1 = wgt.tile([P, 4], f32, name="n1", tag="n1")
        vT1 = wgt.tile([8, H], fp16, name="vT1", tag="vT1")
        v1a = scr.tile([P, 32], f32, name="v1a", tag="v1a")
        mul(v1a, e1n, sm("md1n"))
        v1b = scr.tile([P, 32], f32, name="v1b", tag="v1b")
        mul(v1b, sm("w1n"), sm("mo1n"))
        tt(v1n, v1a, v1b, op=ALU.add)
        for c in range(4):
            sq1 = scr.tile([P, 8], f32, name=f"sq1_{c}", tag="sq1")
            stt(sq1, v1n[:, 8 * c:8 * c + 8], 0, v1n[:, 8 * c:8 * c + 8],
                op0=ALU.bypass, op1=ALU.mult, accum_out=n1[:, c:c + 1])
        s1 = make_scale(n1, eg[1], (P, 4), "l1")
        e1s = wgt.tile([P, 4], f32, name="e1s", tag="e1s")
        mul(e1s, e1d, s1)
        for c in range(4):
            pt = pf.tile([8, P], f32, name=f"pt1_{c}", tag="pt1")
            nc.tensor.transpose(pt, v1n[:, 8 * c:8 * c + 8], ident)
            cp(vT1[:, P * c:P * c + P], pt)

        # ======= layer 4 prep (early: only needs smalls/esm) ==============
        v4a = scr.tile([P, 32], f32, name="v4a", tag="v4a")
        mul(v4a, e4t, sm("md4t"))
        v4b = scr.tile([P, 32], f32, name="v4b", tag="v4b")
        mul(v4b, sm("w4t"), sm("mo4t"))
        vt4 = wgt.tile([P, 32], fp16, name="vt4", tag="vt4")
        tt(vt4, v4a, v4b, op=ALU.add)
        vsq4 = scr.tile([P, 32], fp16, name="vsq4", tag="vsq4")
        mul(vsq4, vt4, vt4)
        n4 = pn.tile([1, 8], f32, name="n4", tag="pn")
        for k in range(4):
            mm(n4, ones4[:, 0:1], vsq4[:, 8 * k:8 * k + 8],
               start=(k == 0), stop=(k == 3))
        s4r = make_scale(n4, eg4row, (1, 8), "l4")
        s4b = wgt.tile([P, 16], f32, name="s4b", tag="s4b")
        nc.gpsimd.partition_broadcast(s4b[:, 0:8], s4r)
        nc.gpsimd.partition_broadcast(s4b[:, 8:16], s4r)
        # G-flow layer-4 stationary: exp(W4 diag blocks); s3 already in G3
        vd4 = wgt.tile([P, 32], bf16, name="vd4", tag="vd4")
        cp(vd4, v4a)

        # ================= layer 1 batch ==================================
        h1 = wgt.tile([P, 4 * BC], fp16, name="h1", tag="h1")
        G1 = wgt.tile([P, 4 * BC], bf16, name="G1", tag="G1")
        pz1 = pz.tile([P, 4 * BC], f32, name="pz1", tag="pz")
        for c in range(4):
            cs = slice(BC * c, BC * c + BC)
            mm(pz1[:, cs], vT1[:, P * c:P * c + P], xTt)
            act(h1[:, cs], pz1[:, cs], E.Tanh,
                bias=sm("b1")[:, c:c + 1], scale=s1[:, c:c + 1])
            hqc = scr.tile([P, BC], fp16, name=f"hq1_{c}", tag="hqc")
            mul(hqc, h1[:, cs], h1[:, cs])
            scc = scr.tile([P, BC], fp16, name=f"sc1_{c}", tag="scc")
            ts(scc, hqc, -1.0, 1.0, op0=ALU.mult, op1=ALU.add)
            ts(G1[:, cs], scc, e1s[:, c:c + 1], None, op0=ALU.mult)

        # ================= layer 2/3 prep ================================
        def prep_stat(l, vt):
            # in-place exp of the 8 diag 64x64 blocks: two strided ACT ops
            dA = vt[0:64, 0:H].rearrange("p (b c) -> p b c", c=128)[:, :, 0:64]
            act(dA, dA, E.Exp)
            dB = vt[64:128, 0:H].rearrange("p (b c) -> p b c", c=128)[:, :, 64:128]
            act(dB, dB, E.Exp)
            # G-flow stationary: bf16 copy of diag strip (gpsimd), UR zeroed
            Gd = wgt.tile([P, H], bf16, name=f"Gd{l}", tag=f"Gd{l}")
            nc.gpsimd.tensor_copy(Gd, vt[:, 0:H])
            GdUR = Gd[0:64, :].rearrange("p (b c) -> p b c", c=128)[:, :, 64:128]
            nc.gpsimd.memset(GdUR, 0.0)
            # vsq over the whole packed tile (diag strip is post-exp)
            vsq = scr.tile([P, 1280], fp16, name=f"vsq{l}", tag="vsq")
            mul(vsq, vt, vt)
            return Gd, vsq

        def prep_norms(l, vsq):
            # norm2 row: ones-stationary matmuls -> [1, 512] psum
            nrow = pn.tile([1, H], f32, name=f"nrow{l}", tag="pn")
            mm(nrow, ones4[:, 0:1], vsq[:, 0:H], start=True, stop=False,
               skip_group_check=True)
            for k in range(3):
                w = 384 - 128 * k
                mm(nrow[:, P * (k + 1):H], ones4[:, 0:1],
                   vsq[:, OFF2[k]:OFF2[k] + w],
                   start=False, stop=(k == 2), skip_group_check=True)
            nrS = scr.tile([1, H], f32, name=f"nrS{l}", tag="nrS")
            act(nrS, nrow, E.Copy)
            # columnize: four tiny PE transposes [1,128] -> [128,1]
            tp = pn.tile([P, 4], f32, name=f"tp{l}", tag="pn")
            for q in range(4):
                nc.tensor.transpose(tp[:, q:q + 1], nrS[0:1, P * q:P * q + P],
                                    ident[0:1, 0:1])
            return make_scale(tp, eg[l], (P, 4), f"l{l}")

        Gd2, vsq2 = prep_stat(2, vt2)
        s2 = prep_norms(2, vsq2)
        Gd3, vsq3 = prep_stat(3, vt3)

        # ================= layer 2/3 batch ================================
        def big_batch(l, vt, Gd, s, h_prev, G_prev):
            hl = wgt.tile([P, 4 * BC], fp16, name=f"h{l}", tag=f"h{l}")
            Gl = wgt.tile([P, 4 * BC], bf16, name=f"G{l}", tag=f"G{l}")
            pzl = pz.tile([P, 4 * BC], f32, name=f"pz{l}", tag="pz")
            pfl = pf.tile([P, 4 * BC], f32, name=f"pf{l}", tag="pf")
            for c in range(4):
                cs = slice(BC * c, BC * c + BC)
                zc = pzl[:, cs]
                for k in range(c + 1):
                    mm(zc, _vsl(vt, k, c), h_prev[:, BC * k:BC * k + BC],
                       start=(k == 0), stop=(k == c))
                act(hl[:, cs], zc, E.Tanh,
                    bias=sm(f"b{l}")[:, c:c + 1], scale=s[:, c:c + 1])
                mm(pfl[:, cs], Gd[:, P * c:P * c + P], G_prev[:, cs])
                # per-chunk G pipeline: sech2 + s-fold, right behind the tanh
                hqc = scr.tile([P, BC], fp16, name=f"hq{l}_{c}", tag="hqc")
                mul(hqc, hl[:, cs], hl[:, cs])
                scc = scr.tile([P, BC], fp16, name=f"sc{l}_{c}", tag="scc")
                ts(scc, hqc, -1.0, 1.0, op0=ALU.mult, op1=ALU.add)
                stt(Gl[:, cs], pfl[:, cs], s[:, c:c + 1], scc,
                    op0=ALU.mult, op1=ALU.mult)
            return hl, Gl

        h2, G2 = big_batch(2, vt2, Gd2, s2, h1, G1)

        s3 = prep_norms(3, vsq3)

        h3, G3 = big_batch(3, vt3, Gd3, s3, h2, G2)

        # ================= layer 4 batch (fully transposed) ===============
        z4 = pn.tile([P, 16], f32, name="z4", tag="pn")
        for b in range(2):
            for k in range(4):
                mm(z4[:, 8 * b:8 * b + 8],
                   h3[:, BC * k + P * b:BC * k + P * b + P],
                   vt4[:, 8 * k:8 * k + 8], start=(k == 0), stop=(k == 3))
        # scale/bias in the transposed layout (per free-col): z*s4 + b4
        z4s = scr.tile([P, 16], f32, name="z4s", tag="z4s")
        mul(z4s, z4, s4b)
        z4t = scr.tile([P, 16], f32, name="z4t", tag="z4t")
        tt(z4t, z4s, sm("b4rep"), op=ALU.add)
        h4 = wgt.tile([P, 16], f32, name="h4", tag="h4")
        act(h4, z4t, E.Tanh)
        nc.sync.dma_start(t["h4T_out"], h4)
        p4 = pf.tile([P, 16], f32, name="p4", tag="pf")
        for b in range(2):
            for k in range(4):
                mm(p4[:, 8 * b:8 * b + 8],
                   G3[:, BC * k + P * b:BC * k + P * b + P],
                   vd4[:, 8 * k:8 * k + 8], start=(k == 0), stop=(k == 3))
        hq4 = scr.tile([P, 16], f32, name="hq4", tag="hq4")
        mul(hq4, h4, h4)
        s24 = scr.tile([P, 16], f32, name="s24", tag="s24")
        ts(s24, hq4, -1.0, 1.0, op0=ALU.mult, op1=ALU.add)
        # gt = s4 * p4 * (1 - h4^2); all positive
        gp = scr.tile([P, 16], f32, name="gp", tag="gp")
        mul(gp, p4, s4b)
        gt = wgt.tile([P, 16], f32, name="gt", tag="gt")
        mul(gt, gp, s24)
        # fast log: ln(x) ~= LN2_A * float(bits(x)) + LN2_B
        gf = scr.tile([P, 16], f32, name="gf", tag="gf")
        cp(gf, gt.bitcast(u32))
        sld = wgt.tile([P, 16], f32, name="sld", tag="sld")
        ts(sld, gf, LN2_A, LN2_B, op0=ALU.mult, op1=ALU.add)
        nc.sync.dma_start(t["sldT_out"], sld)

    nc.compile()
    return nc


def _host_prep(x, W1, logg1, bias1, W2, logg2, bias2, W3, logg3, bias3,
               W4, logg4, bias4):
    """Pure layout prep (transpose / reshape / gather / masks), no arithmetic."""
    f = np.float32

    def cols(a):          # [512]-ish vector -> [128, 4] column-chunk layout
        return np.ascontiguousarray(np.reshape(a, (4, P)).T).astype(f)

    def fold(m):          # [512, 8] -> [128, (k x)] with k = row-chunk
        return m.reshape(4, P, 8).transpose(1, 0, 2).reshape(P, 32)

    def pack(WT):         # [512, 512] W.T -> [128, 1280] diag strip + windows
        wp = np.empty((P, 1280), f)
        for k in range(4):
            d = np.array(WT[P * k:P * k + P, P * k:P * k + P])
            d[64:128, 0:64] = 0.0          # structural mask: LL quadrant
            wp[:, P * k:P * k + P] = d
        for k in range(3):
            w = 384 - 128 * k
            wp[:, OFF2[k]:OFF2[k] + w] = WT[P * k:P * k + P, P * (k + 1):H]
        return wp

    smalls = np.zeros((P, SMALL_W), f)

    def put(name, arr):
        a, b = _SM[name]
        smalls[:arr.shape[0], a:b] = arr

    put("ident", np.eye(P, dtype=f))
    put("w1n", fold(np.asarray(W1)))                       # natural [512,8]
    put("w4t", fold(np.ascontiguousarray(np.asarray(W4).T)))  # [512,8]
    put("w1dg", cols(W1[np.arange(H), np.arange(H) // 64]))
    put("lg1", cols(logg1)); put("b1", cols(bias1))
    put("lg2", cols(logg2)); put("b2", cols(bias2))
    put("lg3", cols(logg3)); put("b3", cols(bias3))
    smalls[0, _SM["lg4r"][0]:_SM["lg4r"][0] + 8] = np.asarray(logg4).reshape(8)
    put("b4rep", np.broadcast_to(
        np.concatenate([np.asarray(bias4).reshape(8)] * 2).reshape(1, 16),
        (P, 16)))
    # structural masks
    o = np.arange(H)[:, None] // 64
    i1 = np.arange(8)[None, :]
    md1 = (i1 == o).astype(f); mo1 = (i1 < o).astype(f)    # [512, 8] natural
    put("md1n", fold(md1)); put("mo1n", fold(mo1))
    ii = np.arange(H)[:, None] // 64
    o4 = np.arange(8)[None, :]
    md4 = (o4 == ii).astype(f); mo4 = (o4 > ii).astype(f)  # [512, 8] W4.T
    put("md4t", fold(md4)); put("mo4t", fold(mo4))

    wp2 = pack(np.ascontiguousarray(np.asarray(W2).T).astype(f))
    wp3 = pack(np.ascontiguousarray(np.asarray(W3).T).astype(f))
    xT = np.ascontiguousarray(np.asarray(x).T).astype(f)   # [8, 2048]
    return xT, wp2, wp3, smalls


def kernel(**inputs):
    global LAST_RESULTS
    from concourse.bass_utils import run_bass_kernel_spmd

    xT, wp2, wp3, smalls = _host_prep(**{k: np.asarray(v) for k, v in inputs.items()})

    if "nc" not in _CACHE:
        _CACHE["nc"] = _build()
    nc = _CACHE["nc"]

    in_maps = []
    for c in range(NCORE):
        in_maps.append({
            "xT": np.ascontiguousarray(xT[:, BC * c:BC * (c + 1)]),
            "wp2": wp2, "wp3": wp3, "smalls": smalls,
        })
    res = run_bass_kernel_spmd(nc, in_maps, core_ids=list(range(NCORE)),
                               trace=TRACE)
    LAST_RESULTS = res

    B = BC * NCORE
    h = np.empty((B, 8), np.float32)
    sld = np.empty((B, 8), np.float32)
    for c, r in enumerate(res.results):
        h4 = r["h4T_out"]          # [128, 16]: h[128b+p, o] = h4[p, 8b+o]
        sl = r["sldT_out"]
        for b in range(2):
            h[BC * c + P * b: BC * c + P * (b + 1)] = h4[:, 8 * b:8 * b + 8]
            sld[BC * c + P * b: BC * c + P * (b + 1)] = sl[:, 8 * b:8 * b + 8]
    return h, sld


# revision 23
# speedup vs baseline: 1.4769x; 1.0133x over previous
"""BNAF forward (B=2048, D=8, H=512, 4 masked layers) on 8 TRN2 NeuronCores.

Strategy
--------
Pure data parallel: batch is split 256/core; the small weights are replicated.

Math: the BNAF log-det recursion collapses in exp space.  For each masked
linear layer, exp(logdet diag blocks) == the diag blocks of the normalized
weight w itself, and for tanh, exp(logdet) == 1 - h^2.  So the whole
log-sum-exp flow is a chain of *positive* block-diagonal matmuls with one
log() at the very end.  The per-output norm scale s = exp(logg)/||v|| is
folded into the G-flow moving operand (G_in = s * G), so the G-flow
stationary is just exp(W) block-diag.

v3 layout/schedule:
- big-layer weights land as packed [128,1280] f32 HWDGE DMAs (diag strip
  first: [[Wd_A, UR],[0, Wd_B]] per chunk with host-zeroed LL, then the
  strictly-lower windows), one per layer, on the two HW DGE rings.
- gpsimd only does 2 big fp32->fp16 casts + the hq/sech2 elementwise pairs
  (every op on the Q7 costs ~0.4us fixed, so no small ops there).
- diag exp is TWO in-place strided ACT ops per layer (dA strip, dB strip);
  the G-flow stationary is a bf16 cast of the exp'd diag strip with UR
  quadrants zeroed (4 DVE memsets).
- per-out-feature norms: ones-stationary window matmuls -> [1,512] psum row
  -> DVE copy -> 4 tiny PE transposes -> [128,4] -> DVE Newton rsqrt.
- layer 4 runs fully transposed (stationary = h3/G3 batch-halves, moving =
  the tiny [128,8] layer-4 weights) producing [128,16] tiles: batch on
  partitions, so the tail elementwise chain + final log run on 128 lanes.
- final Ln is a 2-op DVE fast-log (bitcast + affine), no second ACT table.
- ACT only ever uses {Exp, Tanh}: single table load at kernel start.
"""

import numpy as np

TRACE = False          # set by test.py for profiling runs
LAST_RESULTS = None    # BassKernelResults stash for test.py

_CACHE = {}

P = 128
BC = 256          # batch per core
H = 512
NCORE = 8
MAGIC = 0x5f3759df
OFF2 = (512, 896, 1152)   # packed col offsets of the strictly-lower windows
LN2_A = 8.262958294867817e-08     # ln2 * 2^-23
LN2_B = -87.99988660234897        # -126.9570 * ln2

# smalls layout: first the exp block (exp'd in one ACT op), then the rest.
# wNmd* entries hold where(mask_d, W, -100): exp gives exp(W)*mask_d exactly,
# so the masked-linear weights need no on-device mask multiplies.
_SM = {}
_off = 0
for _name, _w in [("w1dg", 4), ("lg1", 4), ("lg2", 4), ("lg3", 4),
                  ("lg4r", 8), ("w1mdN", 32), ("w4mdT", 32),      # exp block
                  ("b1", 4), ("b2", 4), ("b3", 4), ("b4rep", 16),
                  ("w1moN", 32), ("w4moT", 32), ("ident", 128)]:
    _SM[_name] = (_off, _off + _w)
    _off += _w
SMALL_W = _off
EXPW = _SM["w4mdT"][1]        # width of the exp block (88)


def _vsl(vt, k, c):
    """Packed-layout slice of the (in-chunk k, out-chunk c) 128x128 block."""
    if k == c:
        return vt[:, 128 * k:128 * k + 128]
    o = OFF2[k] + 128 * (c - k - 1)
    return vt[:, o:o + 128]


def _build():
    import concourse.bacc as bacc
    import concourse.mybir as mybir
    import concourse.tile as tile
    from contextlib import ExitStack

    f32 = mybir.dt.float32
    u32 = mybir.dt.uint32
    bf16 = mybir.dt.bfloat16
    fp16 = mybir.dt.float16
    E = mybir.ActivationFunctionType
    ALU = mybir.AluOpType

    nc = bacc.Bacc("TRN2", target_bir_lowering=False, debug=False,
                   enable_asserts=False, num_devices=NCORE)

    t = {}
    t["xT"] = nc.dram_tensor("xT", (8, BC), f32, kind="ExternalInput").ap()
    t["wp2"] = nc.dram_tensor("wp2", (P, 1280), f32, kind="ExternalInput").ap()
    t["wp3"] = nc.dram_tensor("wp3", (P, 1280), f32, kind="ExternalInput").ap()
    t["w1s"] = nc.dram_tensor("w1s", (8, 1024), f32, kind="ExternalInput").ap()
    t["smalls"] = nc.dram_tensor("smalls", (P, SMALL_W), f32, kind="ExternalInput").ap()
    t["h4T_out"] = nc.dram_tensor("h4T_out", (P, 16), f32, kind="ExternalOutput").ap()
    t["sldT_out"] = nc.dram_tensor("sldT_out", (P, 16), f32, kind="ExternalOutput").ap()

    def mm(out, lhsT, rhs, **kw):
        nc.tensor.matmul(out, lhsT, rhs, **kw)

    with tile.TileContext(nc) as tc, ExitStack() as ctx:
        wgt = ctx.enter_context(tc.tile_pool(name="wgt", bufs=1))
        scr = ctx.enter_context(tc.tile_pool(name="scr", bufs=3))
        pz = ctx.enter_context(tc.tile_pool(name="pz", bufs=2, space="PSUM"))
        pf = ctx.enter_context(tc.tile_pool(name="pf", bufs=1, space="PSUM"))
        pn = ctx.enter_context(tc.tile_pool(name="pn", bufs=1, space="PSUM"))

        act = nc.scalar.activation
        cp = nc.vector.tensor_copy
        ts = nc.vector.tensor_scalar
        stt = nc.vector.scalar_tensor_tensor
        mul = nc.vector.tensor_mul
        tt = nc.vector.tensor_tensor

        # ---- input DMAs: smalls on HWDGE; x + packed weights as SWDGE
        # cast-DMAs (fp32 DRAM -> fp16 SBUF, cast inline in the SDMA) ----
        smalls = wgt.tile([P, SMALL_W], f32, name="smalls_t", tag="smalls_t")
        nc.sync.dma_start(smalls, t["smalls"])
        w1s = wgt.tile([8, 1024], f32, name="w1s_t", tag="w1s_t")
        nc.sync.dma_start(w1s, t["w1s"])
        vt2 = wgt.tile([P, 1280], fp16, name="vt2", tag="vt2")
        nc.gpsimd.dma_start(vt2, t["wp2"])
        xTt = wgt.tile([8, BC], fp16, name="xTt", tag="xTt")
        nc.gpsimd.dma_start(xTt, t["xT"])
        vt3 = wgt.tile([P, 1280], fp16, name="vt3", tag="vt3")
        nc.gpsimd.dma_start(vt3, t["wp3"])

        def sm(name):
            a, b = _SM[name]
            return smalls[:, a:b]

        ident = sm("ident")

        # ---- tiny constants on DVE (keep Q7 free) ----
        magict = wgt.tile([P, 8], u32, name="magict", tag="magict")
        nc.vector.memset(magict, MAGIC)
        ones4f = wgt.tile([P, 4], f32, name="ones4f", tag="ones4f")
        nc.vector.memset(ones4f, 1.0)
        ones4 = wgt.tile([P, 4], fp16, name="ones4", tag="ones4")
        cp(ones4, ones4f)
        wz = wgt.tile([P, BC], fp16, name="wz", tag="wz")
        nc.vector.memset(wz, 0.0)

        # short PE warm-up burst (HAM un-throttle) while DMAs drain
        pw = pn.tile([2, BC - 2], f32, name="pw", tag="pn")
        for _ in range(8):
            mm(pw, wz[:, 0:2], wz[:, 2:BC], skip_group_check=True)

        # one batched exp over the whole exp block
        esm = wgt.tile([P, EXPW], f32, name="esm", tag="esm")
        act(esm, smalls[:, 0:EXPW], E.Exp)

        def esl(name):
            a, b = _SM[name]
            return esm[:, a:b]

        e1d = esl("w1dg")
        eg = {1: esl("lg1"), 2: esl("lg2"), 3: esl("lg3")}
        eg4row = esm[0:1, _SM["lg4r"][0]:_SM["lg4r"][0] + 8]
        e1mdN = esl("w1mdN")
        e4mdT = esl("w4mdT")
        # exp of the masked-diag W1.T row block [8, 512]
        e1T = wgt.tile([8, H], f32, name="e1T", tag="e1T")
        act(e1T, w1s[:, 0:H], E.Exp)

        # s = eg * rsqrt(norm2): DVE-only Newton rsqrt
        def make_scale(n2_ap, eg_ap, shape, nm):
            pr = shape[0]
            n2s = scr.tile(list(shape), f32, name=f"n2s_{nm}", tag="sc_n2s")
            cp(n2s, n2_ap)
            shf = scr.tile(list(shape), u32, name=f"shf_{nm}", tag="sc_shf")
            ts(shf, n2s.bitcast(u32), 1, None, op0=ALU.arith_shift_right)
            y0 = scr.tile(list(shape), u32, name=f"y0_{nm}", tag="sc_y0")
            stt(y0, magict[:pr, :shape[1]], 0, shf, op0=ALU.bypass, op1=ALU.subtract)
            y = y0.bitcast(f32)
            t1 = scr.tile(list(shape), f32, name=f"t1_{nm}", tag="sc_t1")
            t2 = scr.tile(list(shape), f32, name=f"t2_{nm}", tag="sc_t2")
            for it in range(1):         # one Newton step: y *= 1.5 - 0.5*n2*y*y
                mul(t1, y, y)
                mul(t2, t1, n2s)
                ts(t1, t2, -0.5, 1.5, op0=ALU.mult, op1=ALU.add)
                yn = scr.tile(list(shape), f32, name=f"yn{it}_{nm}", tag=f"sc_yn{it}")
                mul(yn, y, t1)
                y = yn
            s = wgt.tile(list(shape), f32, name=f"s_{nm}", tag=f"s_{nm}")
            mul(s, eg_ap, y)
            return s

        # ================= layer 1 prep ===================================
        # v1n (natural fold, for norms) and v1T (stationary): one add each
        v1n = wgt.tile([P, 32], f32, name="v1n", tag="v1n")
        tt(v1n, e1mdN, sm("w1moN"), op=ALU.add)
        n1 = wgt.tile([P, 4], f32, name="n1", tag="n1")
        for c in range(4):
            sq1 = scr.tile([P, 8], f32, name=f"sq1_{c}", tag="sq1")
            stt(sq1, v1n[:, 8 * c:8 * c + 8], 0, v1n[:, 8 * c:8 * c + 8],
                op0=ALU.bypass, op1=ALU.mult, accum_out=n1[:, c:c + 1])
        s1 = make_scale(n1, eg[1], (P, 4), "l1")
        e1s = wgt.tile([P, 4], f32, name="e1s", tag="e1s")
        mul(e1s, e1d, s1)
        v1T = wgt.tile([8, H], fp16, name="v1T", tag="v1T")
        tt(v1T, e1T, w1s[:, H:2 * H], op=ALU.add)

        # ======= layer 4 prep (early: only needs smalls/esm) ==============
        vt4 = wgt.tile([P, 32], fp16, name="vt4", tag="vt4")
        tt(vt4, e4mdT, sm("w4moT"), op=ALU.add)
        vsq4 = scr.tile([P, 32], fp16, name="vsq4", tag="vsq4")
        mul(vsq4, vt4, vt4)
        n4 = pn.tile([1, 8], f32, name="n4", tag="pn")
        for k in range(4):
            mm(n4, ones4[:, 0:1], vsq4[:, 8 * k:8 * k + 8],
               start=(k == 0), stop=(k == 3))
        s4r = make_scale(n4, eg4row, (1, 8), "l4")
        s4b = wgt.tile([P, 16], f32, name="s4b", tag="s4b")
        nc.gpsimd.partition_broadcast(s4b[:, 0:8], s4r)
        nc.gpsimd.partition_broadcast(s4b[:, 8:16], s4r)
        # G-flow layer-4 stationary: exp(W4 diag blocks); s3 already in G3
        vd4 = wgt.tile([P, 32], bf16, name="vd4", tag="vd4")
        cp(vd4, e4mdT)

        # ================= layer 1 batch ==================================
        h1 = wgt.tile([P, 4 * BC], fp16, name="h1", tag="h1")
        G1 = wgt.tile([P, 4 * BC], bf16, name="G1", tag="G1")
        pz1 = pz.tile([P, 4 * BC], f32, name="pz1", tag="pz")
        for c in range(4):
            cs = slice(BC * c, BC * c + BC)
            mm(pz1[:, cs], v1T[:, P * c:P * c + P], xTt)
            act(h1[:, cs], pz1[:, cs], E.Tanh,
                bias=sm("b1")[:, c:c + 1], scale=s1[:, c:c + 1])
            hqc = scr.tile([P, BC], fp16, name=f"hq1_{c}", tag="hqc")
            mul(hqc, h1[:, cs], h1[:, cs])
            scc = scr.tile([P, BC], fp16, name=f"sc1_{c}", tag="scc")
            ts(scc, hqc, -1.0, 1.0, op0=ALU.mult, op1=ALU.add)
            ts(G1[:, cs], scc, e1s[:, c:c + 1], None, op0=ALU.mult)

        # ================= layer 2/3 prep ================================
        def prep_stat(l, vt):
            # in-place exp of the 8 diag 64x64 blocks: two strided ACT ops
            dA = vt[0:64, 0:H].rearrange("p (b c) -> p b c", c=128)[:, :, 0:64]
            act(dA, dA, E.Exp)
            dB = vt[64:128, 0:H].rearrange("p (b c) -> p b c", c=128)[:, :, 64:128]
            act(dB, dB, E.Exp)
            # G-flow stationary: bf16 copy of diag strip (gpsimd), UR zeroed
            Gd = wgt.tile([P, H], bf16, name=f"Gd{l}", tag=f"Gd{l}")
            nc.gpsimd.tensor_copy(Gd, vt[:, 0:H])
            GdUR = Gd[0:64, :].rearrange("p (b c) -> p b c", c=128)[:, :, 64:128]
            nc.gpsimd.memset(GdUR, 0.0)
            # vsq over the whole packed tile (diag strip is post-exp)
            vsq = scr.tile([P, 1280], fp16, name=f"vsq{l}", tag="vsq")
            mul(vsq, vt, vt)
            return Gd, vsq

        def prep_norms(l, vsq):
            # norm2 row: ones-stationary matmuls -> [1, 512] psum
            nrow = pn.tile([1, H], f32, name=f"nrow{l}", tag="pn")
            mm(nrow, ones4[:, 0:1], vsq[:, 0:H], start=True, stop=False,
               skip_group_check=True)
            for k in range(3):
                w = 384 - 128 * k
                mm(nrow[:, P * (k + 1):H], ones4[:, 0:1],
                   vsq[:, OFF2[k]:OFF2[k] + w],
                   start=False, stop=(k == 2), skip_group_check=True)
            nrS = scr.tile([1, H], f32, name=f"nrS{l}", tag="nrS")
            act(nrS, nrow, E.Copy)
            # columnize: four tiny PE transposes [1,128] -> [128,1]
            tp = pn.tile([P, 4], f32, name=f"tp{l}", tag="pn")
            for q in range(4):
                nc.tensor.transpose(tp[:, q:q + 1], nrS[0:1, P * q:P * q + P],
                                    ident[0:1, 0:1])
            return make_scale(tp, eg[l], (P, 4), f"l{l}")

        Gd2, vsq2 = prep_stat(2, vt2)
        s2 = prep_norms(2, vsq2)
        Gd3, vsq3 = prep_stat(3, vt3)

        # ================= layer 2/3 batch ================================
        def big_batch(l, vt, Gd, s, h_prev, G_prev):
            hl = wgt.tile([P, 4 * BC], fp16, name=f"h{l}", tag=f"h{l}")
            Gl = wgt.tile([P, 4 * BC], bf16, name=f"G{l}", tag=f"G{l}")
            pzl = pz.tile([P, 4 * BC], f32, name=f"pz{l}", tag="pz")
            pfl = pf.tile([P, 4 * BC], f32, name=f"pf{l}", tag="pf")
            for c in range(4):
                cs = slice(BC * c, BC * c + BC)
                zc = pzl[:, cs]
                for k in range(c + 1):
                    mm(zc, _vsl(vt, k, c), h_prev[:, BC * k:BC * k + BC],
                       start=(k == 0), stop=(k == c))
                act(hl[:, cs], zc, E.Tanh,
                    bias=sm(f"b{l}")[:, c:c + 1], scale=s[:, c:c + 1])
                mm(pfl[:, cs], Gd[:, P * c:P * c + P], G_prev[:, cs])
                # per-chunk G pipeline: sech2 + s-fold, right behind the tanh
                hqc = scr.tile([P, BC], fp16, name=f"hq{l}_{c}", tag="hqc")
                mul(hqc, hl[:, cs], hl[:, cs])
                scc = scr.tile([P, BC], fp16, name=f"sc{l}_{c}", tag="scc")
                ts(scc, hqc, -1.0, 1.0, op0=ALU.mult, op1=ALU.add)
                stt(Gl[:, cs], pfl[:, cs], s[:, c:c + 1], scc,
                    op0=ALU.mult, op1=ALU.mult)
            return hl, Gl

        h2, G2 = big_batch(2, vt2, Gd2, s2, h1, G1)

        s3 = prep_norms(3, vsq3)

        h3, G3 = big_batch(3, vt3, Gd3, s3, h2, G2)

        # ================= layer 4 batch (fully transposed) ===============
        z4 = pn.tile([P, 16], f32, name="z4", tag="pn")
        for b in range(2):
            for k in range(4):
                mm(z4[:, 8 * b:8 * b + 8],
                   h3[:, BC * k + P * b:BC * k + P * b + P],
                   vt4[:, 8 * k:8 * k + 8], start=(k == 0), stop=(k == 3))
        # scale/bias in the transposed layout (per free-col): z*s4 + b4
        z4s = scr.tile([P, 16], f32, name="z4s", tag="z4s")
        mul(z4s, z4, s4b)
        z4t = scr.tile([P, 16], f32, name="z4t", tag="z4t")
        tt(z4t, z4s, sm("b4rep"), op=ALU.add)
        h4 = wgt.tile([P, 16], f32, name="h4", tag="h4")
        act(h4, z4t, E.Tanh)
        nc.sync.dma_start(t["h4T_out"], h4)
        p4 = pf.tile([P, 16], f32, name="p4", tag="pf")
        for b in range(2):
            for k in range(4):
                mm(p4[:, 8 * b:8 * b + 8],
                   G3[:, BC * k + P * b:BC * k + P * b + P],
                   vd4[:, 8 * k:8 * k + 8], start=(k == 0), stop=(k == 3))
        hq4 = scr.tile([P, 16], f32, name="hq4", tag="hq4")
        mul(hq4, h4, h4)
        s24 = scr.tile([P, 16], f32, name="s24", tag="s24")
        ts(s24, hq4, -1.0, 1.0, op0=ALU.mult, op1=ALU.add)
        # gt = s4 * p4 * (1 - h4^2); all positive
        gp = scr.tile([P, 16], f32, name="gp", tag="gp")
        mul(gp, p4, s4b)
        gt = wgt.tile([P, 16], f32, name="gt", tag="gt")
        mul(gt, gp, s24)
        # fast log: ln(x) ~= LN2_A * float(bits(x)) + LN2_B
        gf = scr.tile([P, 16], f32, name="gf", tag="gf")
        cp(gf, gt.bitcast(u32))
        sld = wgt.tile([P, 16], f32, name="sld", tag="sld")
        ts(sld, gf, LN2_A, LN2_B, op0=ALU.mult, op1=ALU.add)
        nc.sync.dma_start(t["sldT_out"], sld)

    nc.compile()
    return nc


def _host_prep(x, W1, logg1, bias1, W2, logg2, bias2, W3, logg3, bias3,
               W4, logg4, bias4):
    """Pure layout prep (transpose / reshape / gather / masks), no arithmetic."""
    f = np.float32

    def cols(a):          # [512]-ish vector -> [128, 4] column-chunk layout
        return np.ascontiguousarray(np.reshape(a, (4, P)).T).astype(f)

    def fold(m):          # [512, 8] -> [128, (k x)] with k = row-chunk
        return m.reshape(4, P, 8).transpose(1, 0, 2).reshape(P, 32)

    def pack(WT):         # [512, 512] W.T -> [128, 1280] diag strip + windows
        wp = np.empty((P, 1280), f)
        for k in range(4):
            d = np.array(WT[P * k:P * k + P, P * k:P * k + P])
            d[64:128, 0:64] = 0.0          # structural mask: LL quadrant
            wp[:, P * k:P * k + P] = d
        for k in range(3):
            w = 384 - 128 * k
            wp[:, OFF2[k]:OFF2[k] + w] = WT[P * k:P * k + P, P * (k + 1):H]
        return wp

    smalls = np.zeros((P, SMALL_W), f)

    def put(name, arr):
        a, b = _SM[name]
        smalls[:arr.shape[0], a:b] = arr

    put("ident", np.eye(P, dtype=f))
    put("w1dg", cols(W1[np.arange(H), np.arange(H) // 64]))
    put("lg1", cols(logg1)); put("b1", cols(bias1))
    put("lg2", cols(logg2)); put("b2", cols(bias2))
    put("lg3", cols(logg3)); put("b3", cols(bias3))
    smalls[0, _SM["lg4r"][0]:_SM["lg4r"][0] + 8] = np.asarray(logg4).reshape(8)
    put("b4rep", np.broadcast_to(
        np.concatenate([np.asarray(bias4).reshape(8)] * 2).reshape(1, 16),
        (P, 16)))
    # structural masks (pre-applied on the host; -100 marks exp->0 positions)
    o = np.arange(H)[:, None] // 64
    i1 = np.arange(8)[None, :]
    md1 = (i1 == o); mo1 = (i1 < o)                        # [512, 8] natural
    W1n = np.asarray(W1).astype(f)
    put("w1mdN", fold(np.where(md1, W1n, f(-100.0))))
    put("w1moN", fold(np.where(mo1, W1n, f(0.0))))
    W4T = np.ascontiguousarray(np.asarray(W4).T).astype(f)  # [512, 8]
    ii = np.arange(H)[:, None] // 64
    o4 = np.arange(8)[None, :]
    md4 = (o4 == ii); mo4 = (o4 > ii)
    put("w4mdT", fold(np.where(md4, W4T, f(-100.0))))
    put("w4moT", fold(np.where(mo4, W4T, f(0.0))))
    # [8, 1024] row-block: [ where(md1.T, W1.T, -100) | where(mo1.T, W1.T, 0) ]
    W1T = np.ascontiguousarray(W1n.T)                      # [8, 512]
    w1s = np.concatenate([np.where(md1.T, W1T, f(-100.0)),
                          np.where(mo1.T, W1T, f(0.0))], axis=1).astype(f)

    wp2 = pack(np.ascontiguousarray(np.asarray(W2).T).astype(f))
    wp3 = pack(np.ascontiguousarray(np.asarray(W3).T).astype(f))
    xT = np.ascontiguousarray(np.asarray(x).T).astype(f)   # [8, 2048]
    return xT, wp2, wp3, smalls, w1s


def kernel(**inputs):
    global LAST_RESULTS
    from concourse.bass_utils import run_bass_kernel_spmd

    xT, wp2, wp3, smalls, w1s = _host_prep(**{k: np.asarray(v) for k, v in inputs.items()})

    if "nc" not in _CACHE:
        _CACHE["nc"] = _build()
    nc = _CACHE["nc"]

    in_maps = []
    for c in range(NCORE):
        in_maps.append({
            "xT": np.ascontiguousarray(xT[:, BC * c:BC * (c + 1)]),
            "wp2": wp2, "wp3": wp3, "smalls": smalls, "w1s": w1s,
        })
    res = run_bass_kernel_spmd(nc, in_maps, core_ids=list(range(NCORE)),
                               trace=TRACE)
    LAST_RESULTS = res

    B = BC * NCORE
    h = np.empty((B, 8), np.float32)
    sld = np.empty((B, 8), np.float32)
    for c, r in enumerate(res.results):
        h4 = r["h4T_out"]          # [128, 16]: h[128b+p, o] = h4[p, 8b+o]
        sl = r["sldT_out"]
        for b in range(2):
            h[BC * c + P * b: BC * c + P * (b + 1)] = h4[:, 8 * b:8 * b + 8]
            sld[BC * c + P * b: BC * c + P * (b + 1)] = sl[:, 8 * b:8 * b + 8]
    return h, sld
